# revision 4
# baseline (speedup 1.0000x reference)
"""Trainium2 Bass kernel for the HLoss1 histogram-binning entropy loss.

Reference semantics:
    r   = clip(x1 - x2, -2, 2)
    idx = round(r / 0.1) + 20              # one-hot index in [0, 40], always valid
    b   = softmax(one_hot(idx, 41)) * log_softmax(one_hot(idx, 41))
    out = -sum(b) / B

For every element [b, d], idx is a valid index, so one_hot(idx, 41) is a
permutation of the same vector (one 1.0, forty 0.0).  softmax / log_softmax
are permutation-equivariant, so -sum(softmax(v) * log_softmax(v)) is the same
scalar for every element regardless of idx:
    c = log(e + 40) - e / (e + 40)
The loss is therefore exactly constant in x1/x2:
    out = B * D * c / B = D * c        (D = 8192)
This identity holds for ALL inputs, so the memory-optimal kernel moves zero
input bytes: each of the 8 data-parallel cores emits its shard's partial sum
RB * D * c (RB = 256 rows/core) and the host combines sum / B, exactly as the
data-parallel sharding would.

Device program (per core): a single sequencer TENSOR_STORE of the f32
constant to the DRAM output.  The instruction placement is tuned against the
NEFF's fixed scaffolding (startup barriers, const-AP memsets, semaphore-clear
teardown):
  - the output-pointer TENSOR_LOAD issues at body start, overlapping the
    gpsimd const memsets,
  - the TENSOR_STORE sits between the engine's pre-barrier drain (which
    carries the broadcast-arrive) and the barrier release wait, so its
    completion receipt overlaps the exit barrier instead of extending it;
    the multi-microsecond teardown that follows guarantees the posted store
    is complete long before the NEFF finishes.

Sharding: pure data parallel over dim 0 - 8 cores x 256 rows each; the
scalar combine (sum / B) happens on host.
"""

import math
import struct

import numpy as np

import concourse.bacc as bacc
from concourse import mybir
from concourse.bass_utils import run_bass_kernel_spmd

B, D = 2048, 8192
NCORES = 8
RB = B // NCORES          # rows per core (256)

# per-element entropy of a one-hot softmax over 41 levels
C_ENT = math.log(math.e + 40.0) - math.e / (math.e + 40.0)

_CACHE = {}


def _build_tuned():
    nc = bacc.Bacc("TRN2", target_bir_lowering=False, debug=False)
    out = nc.dram_tensor("out", [1, 1], mybir.dt.float32, kind="ExternalOutput").ap()

    blk = nc.main_func.blocks[0]
    n0 = len(blk.instructions)
    val_bits = struct.unpack("<i", struct.pack("<f", float(C_ENT * RB * D)))[0]
    nc.vector.store(out, val_bits)
    ours = blk.instructions[n0:]
    del blk.instructions[n0:]
    mv, ld, st = ours
    assert type(mv).__name__ == "InstRegisterMove", type(mv).__name__
    assert type(ld).__name__ == "InstTensorLoad", type(ld).__name__

    dve = mybir.EngineType.DVE
    didx = next(
        i for i, inst in enumerate(blk.instructions)
        if getattr(inst, "engine", None) == dve
        and type(inst).__name__ == "InstDrain"
    )
    nxt = blk.instructions[didx + 1]
    assert getattr(nxt, "engine", None) == dve
    assert type(nxt).__name__ == "InstEventSemaphore", type(nxt).__name__
    # [load, move] before the drain; store between drain and release wait
    blk.instructions[didx + 1:didx + 1] = [st]
    blk.instructions[didx:didx] = [ld, mv]

    nc.finalize()
    return nc


def _build_plain():
    nc = bacc.Bacc("TRN2", target_bir_lowering=False, debug=False)
    out = nc.dram_tensor("out", [1, 1], mybir.dt.float32, kind="ExternalOutput").ap()
    val_bits = struct.unpack("<i", struct.pack("<f", float(C_ENT * RB * D)))[0]
    nc.vector.store(out, val_bits)
    nc.finalize()
    return nc


def _build_bass():
    try:
        return _build_tuned()
    except Exception:
        # instruction layout differs from the expected emission; the untuned
        # program is ~5% slower but identical in semantics
        return _build_plain()


def _get_bass():
    if "nc" not in _CACHE:
        _CACHE["nc"] = _build_bass()
    return _CACHE["nc"]


def run(x1, x2, **spmd_kwargs):
    """Run the SPMD kernel; returns (scalar result, BassKernelResults)."""
    assert tuple(np.shape(x1)) == (B, D) and tuple(np.shape(x2)) == (B, D)
    nc = _get_bass()
    in_maps = [{} for _ in range(NCORES)]
    res = run_bass_kernel_spmd(nc, in_maps, core_ids=list(range(NCORES)), **spmd_kwargs)
    total = np.sum([r["out"].astype(np.float64) for r in res.results])
    return np.array(total / B, dtype=np.float32), res


def kernel(x1, x2):
    result, _ = run(x1, x2)
    return result


# revision 6
# speedup vs baseline: 1.0120x; 1.0120x over previous
"""Trainium2 Bass kernel for the HLoss1 histogram-binning entropy loss.

Reference semantics:
    r   = clip(x1 - x2, -2, 2)
    idx = round(r / 0.1) + 20              # one-hot index in [0, 40], always valid
    b   = softmax(one_hot(idx, 41)) * log_softmax(one_hot(idx, 41))
    out = -sum(b) / B

For every element [b, d], idx is a valid index, so one_hot(idx, 41) is a
permutation of the same vector (one 1.0, forty 0.0).  softmax / log_softmax
are permutation-equivariant, so -sum(softmax(v) * log_softmax(v)) is the same
scalar for every element regardless of idx:
    c = log(e + 40) - e / (e + 40)
The loss is therefore exactly constant in x1/x2:
    out = B * D * c / B = D * c        (D = 8192)
This identity holds for ALL inputs, so the memory-optimal kernel moves zero
input bytes: each of the 8 data-parallel cores emits its shard's partial sum
RB * D * c (RB = 256 rows/core) and the host combines sum / B, exactly as the
data-parallel sharding would.

Device program (per core): a single sequencer TENSOR_STORE of the f32
constant to the DRAM output.  The instruction placement is tuned against the
NEFF's fixed scaffolding (startup barriers, const-AP memsets, semaphore-clear
teardown):
  - the output-pointer TENSOR_LOAD issues at body start, overlapping the
    gpsimd const memsets,
  - the TENSOR_STORE sits between the engine's pre-barrier drain (which
    carries the broadcast-arrive) and the barrier release wait, so its
    completion receipt overlaps the exit barrier instead of extending it;
    the multi-microsecond teardown that follows guarantees the posted store
    is complete long before the NEFF finishes.

Sharding: pure data parallel over dim 0 - 8 cores x 256 rows each; the
scalar combine (sum / B) happens on host.
"""

import math
import struct

import numpy as np

import concourse.bacc as bacc
from concourse import mybir
from concourse.bass_utils import run_bass_kernel_spmd

B, D = 2048, 8192
NCORES = 8
RB = B // NCORES          # rows per core (256)

# per-element entropy of a one-hot softmax over 41 levels
C_ENT = math.log(math.e + 40.0) - math.e / (math.e + 40.0)

_CACHE = {}


def _build_nobarrier():
    """Fastest layout: the init-barrier broadcast (5x drain + 6x event
    semaphore) that fences the unused const-AP memsets is excised, so the
    vector engine's pointer-load/store chain runs uninterrupted from its
    release into the exit round while the other engines arrive immediately."""
    nc = bacc.Bacc("TRN2", target_bir_lowering=False, debug=False)
    out = nc.dram_tensor("out", [1, 1], mybir.dt.float32, kind="ExternalOutput").ap()
    blk = nc.main_func.blocks[0]
    kill = [i for i, inst in enumerate(blk.instructions)
            if type(inst).__name__ in ("InstDrain", "InstEventSemaphore")]
    assert len(kill) == 11, (len(kill),)
    for i in reversed(kill):
        del blk.instructions[i]
    val_bits = struct.unpack("<i", struct.pack("<f", float(C_ENT * RB * D)))[0]
    nc.vector.store(out, val_bits)
    nc.finalize()
    return nc


def _build_tuned():
    """Fallback layout keeping the init barrier: [load, move] ahead of the
    pre-barrier drain, store between drain and release wait."""
    nc = bacc.Bacc("TRN2", target_bir_lowering=False, debug=False)
    out = nc.dram_tensor("out", [1, 1], mybir.dt.float32, kind="ExternalOutput").ap()

    blk = nc.main_func.blocks[0]
    n0 = len(blk.instructions)
    val_bits = struct.unpack("<i", struct.pack("<f", float(C_ENT * RB * D)))[0]
    nc.vector.store(out, val_bits)
    ours = blk.instructions[n0:]
    del blk.instructions[n0:]
    mv, ld, st = ours
    assert type(mv).__name__ == "InstRegisterMove", type(mv).__name__
    assert type(ld).__name__ == "InstTensorLoad", type(ld).__name__

    dve = mybir.EngineType.DVE
    didx = next(
        i for i, inst in enumerate(blk.instructions)
        if getattr(inst, "engine", None) == dve
        and type(inst).__name__ == "InstDrain"
    )
    nxt = blk.instructions[didx + 1]
    assert getattr(nxt, "engine", None) == dve
    assert type(nxt).__name__ == "InstEventSemaphore", type(nxt).__name__
    # [load, move] before the drain; store between drain and release wait
    blk.instructions[didx + 1:didx + 1] = [st]
    blk.instructions[didx:didx] = [ld, mv]

    nc.finalize()
    return nc


def _build_plain():
    nc = bacc.Bacc("TRN2", target_bir_lowering=False, debug=False)
    out = nc.dram_tensor("out", [1, 1], mybir.dt.float32, kind="ExternalOutput").ap()
    val_bits = struct.unpack("<i", struct.pack("<f", float(C_ENT * RB * D)))[0]
    nc.vector.store(out, val_bits)
    nc.finalize()
    return nc


def _build_bass():
    for builder in (_build_nobarrier, _build_tuned, _build_plain):
        try:
            return builder()
        except Exception:
            # instruction layout differs from the expected emission; the next
            # fallback is slightly slower but identical in semantics
            continue
    raise RuntimeError("all kernel builders failed")


def _get_bass():
    if "nc" not in _CACHE:
        _CACHE["nc"] = _build_bass()
    return _CACHE["nc"]


def run(x1, x2, **spmd_kwargs):
    """Run the SPMD kernel; returns (scalar result, BassKernelResults)."""
    assert tuple(np.shape(x1)) == (B, D) and tuple(np.shape(x2)) == (B, D)
    nc = _get_bass()
    in_maps = [{} for _ in range(NCORES)]
    res = run_bass_kernel_spmd(nc, in_maps, core_ids=list(range(NCORES)), **spmd_kwargs)
    total = np.sum([r["out"].astype(np.float64) for r in res.results])
    return np.array(total / B, dtype=np.float32), res


def kernel(x1, x2):
    result, _ = run(x1, x2)
    return result
